# revision 7
# baseline (speedup 1.0000x reference)
"""Causal self-attention (RoPE quirk variant) on 8 Trainium2 NeuronCores.

Reference computation (B=2, T=1024, C=2048, H=64 heads, hd=32):
  qkv = x @ w_attn; split q,k,v; RoPE(dim=n_head quirk) on q,k;
  causal softmax attention; y @ w_proj.

Sharding: 8 cores = 2 batches x 4 head-quarter shards (16 heads / core).
Each core computes attention for its 16 heads on its batch and a partial
output projection (its 512 channels of the 2048-channel contraction);
the host sums the 4 partials per batch.

Device layout notes:
  * Everything "transposed": qkv^T [chan, T] so the attention matmuls
    (contraction over hd on partitions) and the out-projection
    (contraction over channels on partitions) need no transposes.
  * rotate_half(q) is obtained as an extra projection x @ W_tilde where
    W_tilde is a host-side column-permuted/negated copy of W_q: this makes
    RoPE pure elementwise work on the DVE (no cross-partition shuffles).
  * Scores are built as S^T [T_k, T_q]; softmax denominators via a
    ones-matrix matmul whose output lands already replicated over the 32
    partitions of each head (col-packed 4 heads/instr); causal mask applied
    post-exp with gpsimd affine_select on the diagonal blocks.
  * fp32 data is consumed as float32r (FP22 single-pass matmul, ~13-bit
    mantissa); attention weights and v are bf16.
  * Build order interleaves per-head-group projection -> RoPE -> attention
    so the Tile scheduler can hide ACT exp latency under projection matmuls
    of the next group.
"""

import json
import sys

sys.path.insert(0, "/opt/trn_rl_repo")

import numpy as np

import concourse.bass as bass
import concourse.mybir as mybir
import concourse.tile as tile

F32 = mybir.dt.float32
F32R = mybir.dt.float32r
BF16 = mybir.dt.bfloat16

B, T, C = 2, 1024, 2048
H, HD = 64, 32
SCALE = 1.0 / np.sqrt(32.0)

_PATCHED = False


def _split_excess_waits(bir_json: bytes) -> bytes:
    """The walrus build in this container encodes at most ONE sync-wait per
    instruction; Tile's wait assigner emits several. Hoist excess waits onto
    same-engine NoOps placed immediately before the instruction."""
    d = json.loads(bir_json)
    ctr = 0
    for fn in d.get("functions", []):
        for blk in fn.get("blocks", []):
            out = []
            for inst in blk.get("instructions", []):
                si = inst.get("sync_info")
                waits = (si or {}).get("on_wait") or []
                if len(waits) > 1:
                    for w in waits[:-1]:
                        out.append({
                            "name": f"WSplit-{ctr}",
                            "opcode": "NoOp",
                            "engine": inst["engine"],
                            "ins": [],
                            "outs": [],
                            "sync_info": {"on_update": [], "on_wait": [w]},
                        })
                        ctr += 1
                    si["on_wait"] = [waits[-1]]
                out.append(inst)
            blk["instructions"] = out
    return json.dumps(d).encode()


def _install_patches():
    global _PATCHED
    if _PATCHED:
        return
    import concourse.bass_utils as bu
    import concourse.bass2jax as b2j

    orig = bu.compile_bir_kernel

    def patched_compile(bir_json, tmpdir, neff_name="file.neff"):
        return orig(_split_excess_waits(bir_json), tmpdir, neff_name)

    bu.compile_bir_kernel = patched_compile
    b2j.compile_bir_kernel = patched_compile
    _PATCHED = True


def _build_bass():
    nc = bass.Bass(trn_type="TRN2")
    xT = nc.dram_tensor("xT", [128, 16, 1024], F32R, kind="ExternalInput").ap()
    wA = nc.dram_tensor("wA", [128, 16, 16, 128], F32R, kind="ExternalInput").ap()
    wV = nc.dram_tensor("wV", [128, 16, 512], F32R, kind="ExternalInput").ap()
    wP = nc.dram_tensor("wP", [128, 4, 2048], F32R, kind="ExternalInput").ap()
    cosT = nc.dram_tensor("cosT", [128, 1024], F32R, kind="ExternalInput").ap()
    sinT = nc.dram_tensor("sinT", [128, 1024], F32R, kind="ExternalInput").ap()
    out = nc.dram_tensor("out", [1024, 2048], F32, kind="ExternalOutput").ap()
    outr = out.rearrange("(tq p) n -> tq p n", p=128)

    EXP = mybir.ActivationFunctionType.Exp

    with tile.TileContext(nc) as tc:
        with tc.tile_pool(name="persist", bufs=1) as persist, \
             tc.tile_pool(name="ylate", bufs=1) as ylate:
            # persist: [chan, T] rotated q (dim1 0-3 per group) and k (4-7)
            qkT = persist.tile([128, 8, 1024], F32R)
            v_sb = persist.tile([128, 8, 512], BF16)    # [T_k in blk, kb, chan]
            cos_sb = persist.tile([128, 1024], F32R)
            sin_sb = persist.tile([128, 1024], F32R)
            ones_sb = persist.tile([128, 32], BF16)
            nc.sync.dma_start(cos_sb, cosT)
            nc.sync.dma_start(sin_sb, sinT)
            nc.vector.memset(ones_sb, 1.0)
            y_tiles = [ylate.tile([128, 1024], F32R, tag=f"y{g}", name=f"y{g}")
                       for g in range(4)]

            with tc.tile_pool(name="phA", bufs=1) as xpool, \
                 tc.tile_pool(name="wstream", bufs=2) as wsp:
                xt = xpool.tile([128, 16, 1024], F32R)
                nc.sync.dma_start(xt, xT)

                # ---- v first (all groups): natural layout [T, chan] ----
                with tc.tile_pool(name="psV", bufs=8, space="PSUM") as psv_pool:
                    psv = [psv_pool.tile([128, 512], F32, tag="psv", name=f"psv{i}")
                           for i in range(8)]
                    for ko in range(16):
                        wv = wsp.tile([128, 512], F32R, tag="wv")
                        nc.sync.dma_start(wv, wV[:, ko])
                        for tq in range(8):
                            nc.tensor.matmul(psv[tq],
                                             xt[:, ko, tq * 128:(tq + 1) * 128],
                                             wv, start=(ko == 0), stop=(ko == 15))
                    for tq in range(8):
                        nc.any.tensor_copy(v_sb[:, tq, :], psv[tq])

                # ---- per-group: project q,qt,k,kt -> RoPE -> attention ----
                with tc.tile_pool(name="esp", bufs=3) as esp, \
                     tc.tile_pool(name="ztmp", bufs=1) as ztp, \
                     tc.tile_pool(name="qtp", bufs=2) as qtp, \
                     tc.tile_pool(name="psA", bufs=1, space="PSUM") as psa, \
                     tc.tile_pool(name="psS", bufs=4, space="PSUM") as psS, \
                     tc.tile_pool(name="psYZ", bufs=1, space="PSUM") as psYZ:
                    for g in range(4):
                        # projections: q -> qkT[:,g], qt -> qt_t[:,0],
                        #              k -> qkT[:,4+g], kt -> qt_t[:,1]
                        qt_t = qtp.tile([128, 2, 1024], F32R, tag="qt")
                        dests = [qkT[:, g, :], qt_t[:, 0, :],
                                 qkT[:, 4 + g, :], qt_t[:, 1, :]]
                        for j, mi in enumerate((g, 4 + g, 8 + g, 12 + g)):
                            wt = wsp.tile([128, 16, 128], F32R, tag="wa")
                            nc.sync.dma_start(wt, wA[:, mi])
                            ps0 = psa.tile([128, 512], F32, tag="psA0")
                            ps1 = psa.tile([128, 512], F32, tag="psA1")
                            for ko in range(16):
                                st, sp = ko == 0, ko == 15
                                nc.tensor.matmul(ps0, wt[:, ko, :],
                                                 xt[:, ko, 0:512], start=st, stop=sp)
                                nc.tensor.matmul(ps1, wt[:, ko, :],
                                                 xt[:, ko, 512:1024], start=st, stop=sp)
                            nc.any.tensor_copy(dests[j][:, 0:512], ps0)
                            nc.any.tensor_copy(dests[j][:, 512:1024], ps1)
                        # RoPE in place: qt *= sin; q *= cos; q += qt
                        for base, til in ((qkT[:, g, :], qt_t[:, 0, :]),
                                          (qkT[:, 4 + g, :], qt_t[:, 1, :])):
                            nc.vector.tensor_mul(til, til, sin_sb)
                            nc.vector.tensor_mul(base, base, cos_sb)
                            nc.vector.tensor_add(base, base, til)
                        # attention for this group
                        y_g = y_tiles[g]
                        z_g = ztp.tile([128, 1024], F32, tag="zg")
                        for qc in range(2):
                            q0 = qc * 512
                            nkb = (qc + 1) * 4
                            psy = psYZ.tile([128, 512], F32, tag="psy")
                            psz = psYZ.tile([128, 512], F32, tag="psz")
                            for kb in range(nkb):
                                k0 = kb * 128
                                n0 = max(q0, k0)
                                N = q0 + 512 - n0
                                off = n0 - q0
                                es = esp.tile([128, 4, 512], BF16, tag="es")
                                for h in range(4):
                                    pss = psS.tile([128, 512], F32, tag="pss")
                                    nc.tensor.matmul(
                                        pss[:, :N],
                                        qkT[32 * h:32 * h + 32, 4 + g, k0:k0 + 128],
                                        qkT[32 * h:32 * h + 32, g, n0:n0 + N],
                                        start=True, stop=True,
                                        tile_position=(32 * h, 0))
                                    nc.scalar.activation(es[:, h, :N], pss[:, :N], EXP)
                                if k0 >= q0:
                                    for h in range(4):
                                        nc.gpsimd.affine_select(
                                            es[:, h, 0:128], es[:, h, 0:128],
                                            pattern=[[1, 128]],
                                            channel_multiplier=-1, base=0,
                                            compare_op=mybir.AluOpType.is_ge,
                                            fill=0.0)
                                for h in range(4):
                                    c0 = g * 128 + 32 * h
                                    nc.tensor.matmul(
                                        psy[32 * h:32 * h + 32, off:512],
                                        v_sb[:, kb, c0:c0 + 32],
                                        es[:, h, :N],
                                        start=(kb == 0), stop=(kb == nkb - 1),
                                        tile_position=(0, 32 * h),
                                        skip_group_check=True)
                                    nc.tensor.matmul(
                                        psz[32 * h:32 * h + 32, off:512],
                                        ones_sb,
                                        es[:, h, :N],
                                        start=(kb == 0), stop=(kb == nkb - 1),
                                        tile_position=(0, 32 * h),
                                        skip_group_check=True)
                            nc.any.tensor_copy(y_g[:, q0:q0 + 512], psy)
                            nc.any.tensor_copy(z_g[:, q0:q0 + 512], psz)
                        nc.vector.reciprocal(z_g, z_g)
                        nc.vector.tensor_mul(y_g, y_g, z_g)

            # ---- out projection (wP streamed) ----
            with tc.tile_pool(name="phD", bufs=4) as phd, \
                 tc.tile_pool(name="psO", bufs=8, space="PSUM") as psop:
                wp_t = [phd.tile([128, 4, 512], F32R, tag=f"wp{n}", name=f"wp{n}")
                        for n in range(4)]
                for n in range(4):
                    nc.sync.dma_start(wp_t[n], wP[:, :, n * 512:(n + 1) * 512])
                for tq in range(8):
                    pso = [psop.tile([128, 512], F32, tag="pso", name=f"pso{n}")
                           for n in range(4)]
                    for gk in range(4):
                        lhs = y_tiles[gk][:, tq * 128:(tq + 1) * 128]
                        for n in range(4):
                            nc.tensor.matmul(pso[n], lhs, wp_t[n][:, gk, :],
                                             start=(gk == 0), stop=(gk == 3))
                    for n in range(4):
                        o_sb = phd.tile([128, 512], F32, tag="osb")
                        nc.any.tensor_copy(o_sb, pso[n])
                        nc.sync.dma_start(outr[tq][:, n * 512:(n + 1) * 512], o_sb)
    return nc


_NC_CACHE = None


def _host_inputs(x, pos, w_attn, w_proj):
    """Build the 8 per-core input dicts."""
    x = np.asarray(x, dtype=np.float32)
    pos = np.asarray(pos, dtype=np.float32)
    w_attn = np.asarray(w_attn, dtype=np.float32)
    w_proj = np.asarray(w_proj, dtype=np.float32)

    inv_freq = (1.0 / (10000.0 ** (np.arange(0, H, 2, dtype=np.float32) / H)))
    sinus = pos[:, None] * inv_freq[None, :]              # [T, 32]
    cosT = np.tile(np.cos(sinus).T, (4, 1)).astype(np.float32).copy()  # [128, T]
    sinT = np.tile(np.sin(sinus).T, (4, 1)).astype(np.float32).copy()

    def tilde(w):
        wt = np.empty_like(w)
        wt[:, 0::2] = -w[:, 1::2]
        wt[:, 1::2] = w[:, 0::2]
        return wt

    in_maps = []
    for core in range(8):
        b, g = divmod(core, 4)
        hs = slice(g * 512, (g + 1) * 512)
        Wq = (w_attn[:, 0:2048][:, hs] * SCALE).astype(np.float32)
        Wk = w_attn[:, 2048:4096][:, hs]
        Wv = w_attn[:, 4096:6144][:, hs]
        WA = np.concatenate([Wq, tilde(Wq), Wk, tilde(Wk)], axis=1)  # [2048, 2048]
        wAr = np.ascontiguousarray(
            WA.reshape(16, 128, 16, 128).transpose(1, 2, 0, 3))     # ki mi ko mc
        wVr = np.ascontiguousarray(Wv.reshape(16, 128, 512).transpose(1, 0, 2))
        wPr = np.ascontiguousarray(
            w_proj[hs, :].reshape(4, 128, 2048).transpose(1, 0, 2))
        xTr = np.ascontiguousarray(
            x[b].T.reshape(16, 128, 1024).transpose(1, 0, 2))
        in_maps.append({
            "xT": xTr, "wA": wAr, "wV": wVr, "wP": wPr,
            "cosT": cosT, "sinT": sinT,
        })
    return in_maps


def kernel(x, pos, w_attn, w_proj, _trace=False):
    global _NC_CACHE
    _install_patches()
    from concourse.bass_utils import run_bass_kernel_spmd

    if _NC_CACHE is None:
        _NC_CACHE = _build_bass()
    nc = _NC_CACHE
    in_maps = _host_inputs(x, pos, w_attn, w_proj)
    res = run_bass_kernel_spmd(nc, in_maps, core_ids=list(range(8)), trace=_trace)
    outs = [res.results[c]["out"] for c in range(8)]
    full = np.stack([
        outs[0] + outs[1] + outs[2] + outs[3],
        outs[4] + outs[5] + outs[6] + outs[7],
    ]).astype(np.float32)
    kernel.last_results = res
    return full


# revision 8
# speedup vs baseline: 1.3085x; 1.3085x over previous
"""Causal self-attention (RoPE quirk variant) on 8 Trainium2 NeuronCores.

Reference computation (B=2, T=1024, C=2048, H=64 heads, hd=32):
  qkv = x @ w_attn; split q,k,v; RoPE(dim=n_head quirk) on q,k;
  causal softmax attention; y @ w_proj.

Sharding: 8 cores = 2 batches x 4 head-quarter shards (16 heads / core).
Each core computes attention for its 16 heads on its batch and a partial
output projection (its 512 channels of the 2048-channel contraction);
the host sums the 4 partials per batch.

Device layout notes:
  * Everything "transposed": qkv^T [chan, T] so the attention matmuls
    (contraction over hd on partitions) and the out-projection
    (contraction over channels on partitions) need no transposes.
  * rotate_half is a partition pair-swap done with two partition-strided
    SBUF->SBUF DMAs; the per-channel sign lives in a host-built sin table.
  * Scores are built as S^T [T_k, T_q] in bf16 (row-packed 4 heads);
    softmax denominators via a ones-matrix matmul whose output lands
    already replicated over each head's 32 partitions (col-packed);
    causal mask applied post-exp with gpsimd affine_select on diagonal
    blocks.
  * fp32 data is consumed as float32r (FP22 single-pass matmul); q/k after
    RoPE, attention weights and v are bf16.
  * Build order interleaves per-head-group projection -> RoPE -> attention
    so the Tile scheduler can hide ACT exp latency under projection
    matmuls of the next group.
"""

import json
import sys

sys.path.insert(0, "/opt/trn_rl_repo")

import ml_dtypes
import numpy as np

import concourse.bass as bass
import concourse.mybir as mybir
import concourse.tile as tile

F32 = mybir.dt.float32
F32R = mybir.dt.float32r
BF16 = mybir.dt.bfloat16

B, T, C = 2, 1024, 2048
H, HD = 64, 32
SCALE = 1.0 / np.sqrt(32.0)

_PATCHED = False


def _split_excess_waits(bir_json: bytes) -> bytes:
    """The walrus build in this container encodes at most ONE sync-wait per
    instruction; Tile's wait assigner emits several. Hoist excess waits onto
    same-engine NoOps placed immediately before the instruction."""
    d = json.loads(bir_json)
    ctr = 0
    for fn in d.get("functions", []):
        for blk in fn.get("blocks", []):
            out = []
            for inst in blk.get("instructions", []):
                si = inst.get("sync_info")
                waits = (si or {}).get("on_wait") or []
                if len(waits) > 1:
                    for w in waits[:-1]:
                        out.append({
                            "name": f"WSplit-{ctr}",
                            "opcode": "NoOp",
                            "engine": inst["engine"],
                            "ins": [],
                            "outs": [],
                            "sync_info": {"on_update": [], "on_wait": [w]},
                        })
                        ctr += 1
                    si["on_wait"] = [waits[-1]]
                out.append(inst)
            blk["instructions"] = out
    return json.dumps(d).encode()


def _install_patches():
    global _PATCHED
    if _PATCHED:
        return
    import concourse.bass_utils as bu
    import concourse.bass2jax as b2j

    orig = bu.compile_bir_kernel

    def patched_compile(bir_json, tmpdir, neff_name="file.neff"):
        return orig(_split_excess_waits(bir_json), tmpdir, neff_name)

    bu.compile_bir_kernel = patched_compile
    b2j.compile_bir_kernel = patched_compile
    _PATCHED = True


def _build_bass():
    nc = bass.Bass(trn_type="TRN2")
    xT = nc.dram_tensor("xT", [128, 16, 1024], F32R, kind="ExternalInput").ap()
    wA = nc.dram_tensor("wA", [128, 8, 16, 128], F32R, kind="ExternalInput").ap()
    wV = nc.dram_tensor("wV", [128, 16, 512], F32R, kind="ExternalInput").ap()
    wP = nc.dram_tensor("wP", [128, 4, 2048], F32R, kind="ExternalInput").ap()
    cosT = nc.dram_tensor("cosT", [128, 1024], BF16, kind="ExternalInput").ap()
    sinT = nc.dram_tensor("sinT", [128, 1024], BF16, kind="ExternalInput").ap()
    out = nc.dram_tensor("out", [1024, 2048], F32, kind="ExternalOutput").ap()
    outr = out.rearrange("(tq p) n -> tq p n", p=128)

    EXP = mybir.ActivationFunctionType.Exp

    with tile.TileContext(nc) as tc:
        with tc.tile_pool(name="persist", bufs=1) as persist, \
             tc.tile_pool(name="ylate", bufs=1) as ylate:
            # rotated q (dim1 = group 0-3) and k (4-7), bf16 [chan, T]
            qkT = persist.tile([128, 8, 1024], BF16)
            v_sb = persist.tile([128, 8, 512], BF16)    # [T_k in blk, kb, chan]
            cos_sb = persist.tile([128, 1024], BF16)
            sin_sb = persist.tile([128, 1024], BF16)    # sign-folded
            ones_sb = persist.tile([128, 32], BF16)
            nc.sync.dma_start(cos_sb, cosT)
            nc.sync.dma_start(sin_sb, sinT)
            nc.vector.memset(ones_sb, 1.0)
            y_tiles = [ylate.tile([128, 1024], F32R, tag=f"y{g}", name=f"y{g}")
                       for g in range(4)]

            with tc.tile_pool(name="phA", bufs=1) as xpool, \
                 tc.tile_pool(name="wstream", bufs=2) as wsp:
                xt = xpool.tile([128, 16, 1024], F32R)
                nc.sync.dma_start(xt, xT)

                # ---- v first (all groups): natural layout [T, chan] ----
                with tc.tile_pool(name="psV", bufs=8, space="PSUM") as psv_pool:
                    psv = [psv_pool.tile([128, 512], F32, tag="psv", name=f"psv{i}")
                           for i in range(8)]
                    for ko in range(16):
                        wv = wsp.tile([128, 512], F32R, tag="wv")
                        nc.sync.dma_start(wv, wV[:, ko])
                        for tq in range(8):
                            nc.tensor.matmul(psv[tq],
                                             xt[:, ko, tq * 128:(tq + 1) * 128],
                                             wv, start=(ko == 0), stop=(ko == 15))
                    for tq in range(8):
                        nc.any.tensor_copy(v_sb[:, tq, :], psv[tq])

                # ---- per-group: project q,k -> pair-swap -> RoPE -> attn ----
                with tc.tile_pool(name="esp", bufs=4) as esp, \
                     tc.tile_pool(name="ztmp", bufs=1) as ztp, \
                     tc.tile_pool(name="qtp", bufs=2) as qtp, \
                     tc.tile_pool(name="psA", bufs=1, space="PSUM") as psa, \
                     tc.tile_pool(name="psS", bufs=4, space="PSUM") as psS, \
                     tc.tile_pool(name="psYZ", bufs=1, space="PSUM") as psYZ:
                    for g in range(4):
                        # project q (mi=g) and k (mi=4+g) into pre-RoPE tiles
                        pre = qtp.tile([128, 2, 1024], BF16, tag="pre")
                        swp = qtp.tile([128, 2, 1024], BF16, tag="swp")
                        for j, mi in enumerate((g, 4 + g)):
                            wt = wsp.tile([128, 16, 128], F32R, tag="wa")
                            nc.sync.dma_start(wt, wA[:, mi])
                            ps0 = psa.tile([128, 512], F32, tag="psA0")
                            ps1 = psa.tile([128, 512], F32, tag="psA1")
                            for ko in range(16):
                                st, sp = ko == 0, ko == 15
                                nc.tensor.matmul(ps0, wt[:, ko, :],
                                                 xt[:, ko, 0:512], start=st, stop=sp)
                                nc.tensor.matmul(ps1, wt[:, ko, :],
                                                 xt[:, ko, 512:1024], start=st, stop=sp)
                            nc.any.tensor_copy(pre[:, j, 0:512], ps0)
                            nc.any.tensor_copy(pre[:, j, 512:1024], ps1)
                        # rotate_half channel pair swap via partition-strided DMA
                        pre_v = pre.rearrange("(a b) j f -> a b j f", b=2)
                        swp_v = swp.rearrange("(a b) j f -> a b j f", b=2)
                        nc.sync.dma_start(swp_v[:, 0], pre_v[:, 1])
                        nc.sync.dma_start(swp_v[:, 1], pre_v[:, 0])
                        # RoPE: qkT = pre*cos + swap*sin_signed
                        for j, dst in enumerate((qkT[:, g, :], qkT[:, 4 + g, :])):
                            nc.vector.tensor_mul(swp[:, j, :], swp[:, j, :], sin_sb)
                            nc.vector.tensor_mul(pre[:, j, :], pre[:, j, :], cos_sb)
                            nc.vector.tensor_add(dst, pre[:, j, :], swp[:, j, :])
                        # attention for this group
                        y_g = y_tiles[g]
                        z_g = ztp.tile([128, 1024], F32, tag="zg")
                        for qc in range(2):
                            q0 = qc * 512
                            nkb = (qc + 1) * 4
                            psy = psYZ.tile([128, 512], F32, tag="psy")
                            psz = psYZ.tile([128, 512], F32, tag="psz")
                            for kb in range(nkb):
                                k0 = kb * 128
                                n0 = max(q0, k0)
                                N = q0 + 512 - n0
                                off = n0 - q0
                                es = esp.tile([128, 4, 512], BF16, tag="es")
                                pss = [psS.tile([128, 512], F32, tag="pss",
                                                name=f"pss{g}_{qc}_{kb}_{h}")
                                       for h in range(4)]
                                for h in range(4):
                                    nc.tensor.matmul(
                                        pss[h][:, :N],
                                        qkT[32 * h:32 * h + 32, 4 + g, k0:k0 + 128],
                                        qkT[32 * h:32 * h + 32, g, n0:n0 + N],
                                        start=True, stop=True,
                                        tile_position=(32 * h, 0))
                                for h in range(4):
                                    nc.scalar.activation(es[:, h, :N],
                                                         pss[h][:, :N], EXP)
                                if k0 >= q0:
                                    for h in range(4):
                                        nc.gpsimd.affine_select(
                                            es[:, h, 0:128], es[:, h, 0:128],
                                            pattern=[[1, 128]],
                                            channel_multiplier=-1, base=0,
                                            compare_op=mybir.AluOpType.is_ge,
                                            fill=0.0)
                                for h in range(4):
                                    c0 = g * 128 + 32 * h
                                    nc.tensor.matmul(
                                        psy[32 * h:32 * h + 32, off:512],
                                        v_sb[:, kb, c0:c0 + 32],
                                        es[:, h, :N],
                                        start=(kb == 0), stop=(kb == nkb - 1),
                                        tile_position=(0, 32 * h),
                                        skip_group_check=True)
                                for h in range(4):
                                    nc.tensor.matmul(
                                        psz[32 * h:32 * h + 32, off:512],
                                        ones_sb,
                                        es[:, h, :N],
                                        start=(kb == 0), stop=(kb == nkb - 1),
                                        tile_position=(0, 32 * h),
                                        skip_group_check=True)
                            nc.any.tensor_copy(y_g[:, q0:q0 + 512], psy)
                            nc.any.tensor_copy(z_g[:, q0:q0 + 512], psz)
                        nc.vector.reciprocal(z_g, z_g)
                        nc.vector.tensor_mul(y_g, y_g, z_g)

            # ---- out projection (wP streamed) ----
            with tc.tile_pool(name="phD", bufs=4) as phd, \
                 tc.tile_pool(name="psO", bufs=8, space="PSUM") as psop:
                wp_t = [phd.tile([128, 4, 512], F32R, tag=f"wp{n}", name=f"wp{n}")
                        for n in range(4)]
                for n in range(4):
                    nc.sync.dma_start(wp_t[n], wP[:, :, n * 512:(n + 1) * 512])
                for tq in range(8):
                    pso = [psop.tile([128, 512], F32, tag="pso", name=f"pso{n}")
                           for n in range(4)]
                    for gk in range(4):
                        lhs = y_tiles[gk][:, tq * 128:(tq + 1) * 128]
                        for n in range(4):
                            nc.tensor.matmul(pso[n], lhs, wp_t[n][:, gk, :],
                                             start=(gk == 0), stop=(gk == 3))
                    for n in range(4):
                        o_sb = phd.tile([128, 512], F32, tag="osb")
                        nc.any.tensor_copy(o_sb, pso[n])
                        nc.sync.dma_start(outr[tq][:, n * 512:(n + 1) * 512], o_sb)
    return nc


_NC_CACHE = None


def _host_inputs(x, pos, w_attn, w_proj):
    """Build the 8 per-core input dicts."""
    x = np.asarray(x, dtype=np.float32)
    pos = np.asarray(pos, dtype=np.float32)
    w_attn = np.asarray(w_attn, dtype=np.float32)
    w_proj = np.asarray(w_proj, dtype=np.float32)

    inv_freq = (1.0 / (10000.0 ** (np.arange(0, H, 2, dtype=np.float32) / H)))
    sinus = pos[:, None] * inv_freq[None, :]              # [T, 32]
    cosT = np.tile(np.cos(sinus).T, (4, 1))               # [128, T]
    sinT = np.tile(np.sin(sinus).T, (4, 1)).copy()
    sinT[0::2, :] *= -1.0                                 # rotate_half signs
    cosT = cosT.astype(ml_dtypes.bfloat16)
    sinT = sinT.astype(ml_dtypes.bfloat16)

    in_maps = []
    for core in range(8):
        b, g = divmod(core, 4)
        hs = slice(g * 512, (g + 1) * 512)
        Wq = (w_attn[:, 0:2048][:, hs] * SCALE).astype(np.float32)
        Wk = w_attn[:, 2048:4096][:, hs]
        Wv = w_attn[:, 4096:6144][:, hs]
        WA = np.concatenate([Wq, Wk], axis=1)             # [2048, 1024]
        wAr = np.ascontiguousarray(
            WA.reshape(16, 128, 8, 128).transpose(1, 2, 0, 3))  # ki mi ko mc
        wVr = np.ascontiguousarray(Wv.reshape(16, 128, 512).transpose(1, 0, 2))
        wPr = np.ascontiguousarray(
            w_proj[hs, :].reshape(4, 128, 2048).transpose(1, 0, 2))
        xTr = np.ascontiguousarray(
            x[b].T.reshape(16, 128, 1024).transpose(1, 0, 2))
        in_maps.append({
            "xT": xTr, "wA": wAr, "wV": wVr, "wP": wPr,
            "cosT": cosT, "sinT": sinT,
        })
    return in_maps


def kernel(x, pos, w_attn, w_proj, _trace=False):
    global _NC_CACHE
    _install_patches()
    from concourse.bass_utils import run_bass_kernel_spmd

    if _NC_CACHE is None:
        _NC_CACHE = _build_bass()
    nc = _NC_CACHE
    in_maps = _host_inputs(x, pos, w_attn, w_proj)
    res = run_bass_kernel_spmd(nc, in_maps, core_ids=list(range(8)), trace=_trace)
    outs = [res.results[c]["out"] for c in range(8)]
    full = np.stack([
        outs[0] + outs[1] + outs[2] + outs[3],
        outs[4] + outs[5] + outs[6] + outs[7],
    ]).astype(np.float32)
    kernel.last_results = res
    return full


# revision 23
# speedup vs baseline: 1.6187x; 1.2370x over previous
"""Causal self-attention (RoPE quirk variant) on 8 Trainium2 NeuronCores.

Reference computation (B=2, T=1024, C=2048, H=64 heads, hd=32):
  qkv = x @ w_attn; split q,k,v; RoPE(dim=n_head quirk) on q,k;
  causal softmax attention; y @ w_proj.

Sharding: 8 cores = 2 batches x 4 head-quarter shards (16 heads / core).
Each core computes attention for its 16 heads on its batch and a partial
output projection (its 512 channels of the 2048-channel contraction);
the host sums the 4 partials per batch.

Device layout notes:
  * Everything "transposed": qkv^T [chan, T] so the attention matmuls
    (contraction over hd on partitions) and the out-projection
    (contraction over channels on partitions) need no transposes.
  * rotate_half is a partition pair-swap done with two partition-strided
    SBUF->SBUF DMAs; the per-channel sign lives in a host-built sin table.
  * Scores are built as S^T [T_k, T_q] in bf16 (row-packed 4 heads);
    softmax denominators via a ones-matrix matmul whose output lands
    already replicated over each head's 32 partitions (col-packed);
    causal mask applied post-exp with gpsimd affine_select on diagonal
    blocks.
  * fp32 data is consumed as float32r (FP22 single-pass matmul); q/k after
    RoPE, attention weights and v are bf16.
  * Build order interleaves per-head-group projection -> RoPE -> attention
    so the Tile scheduler can hide ACT exp latency under projection
    matmuls of the next group.
"""

import json
import sys

sys.path.insert(0, "/opt/trn_rl_repo")

import ml_dtypes
import numpy as np

import concourse.bass as bass
import concourse.mybir as mybir
import concourse.tile as tile
from concourse.tile import add_dep_helper

F32 = mybir.dt.float32
F32R = mybir.dt.float32r
BF16 = mybir.dt.bfloat16

B, T, C = 2, 1024, 2048
H, HD = 64, 32
SCALE = 1.0 / np.sqrt(32.0)

_PATCHED = False


def _split_excess_waits(bir_json: bytes) -> bytes:
    """The walrus build in this container encodes at most ONE sync-wait per
    instruction; Tile's wait assigner emits several. Hoist excess waits onto
    same-engine NoOps placed immediately before the instruction."""
    d = json.loads(bir_json)
    ctr = 0
    for fn in d.get("functions", []):
        for blk in fn.get("blocks", []):
            out = []
            for inst in blk.get("instructions", []):
                si = inst.get("sync_info")
                waits = (si or {}).get("on_wait") or []
                if len(waits) > 1:
                    for w in waits[:-1]:
                        out.append({
                            "name": f"WSplit-{ctr}",
                            "opcode": "NoOp",
                            "engine": inst["engine"],
                            "ins": [],
                            "outs": [],
                            "sync_info": {"on_update": [], "on_wait": [w]},
                        })
                        ctr += 1
                    si["on_wait"] = [waits[-1]]
                out.append(inst)
            blk["instructions"] = out
    return json.dumps(d).encode()


def _install_patches():
    global _PATCHED
    if _PATCHED:
        return
    import concourse.bass_utils as bu
    import concourse.bass2jax as b2j

    orig = bu.compile_bir_kernel

    def patched_compile(bir_json, tmpdir, neff_name="file.neff"):
        return orig(_split_excess_waits(bir_json), tmpdir, neff_name)

    bu.compile_bir_kernel = patched_compile
    b2j.compile_bir_kernel = patched_compile
    _PATCHED = True


def _build_bass():
    nc = bass.Bass(trn_type="TRN2")
    xT = nc.dram_tensor("xT", [128, 16, 1024], F32R, kind="ExternalInput").ap()
    wA = nc.dram_tensor("wA", [128, 12, 16, 128], F32R, kind="ExternalInput").ap()
    wP = nc.dram_tensor("wP", [128, 4, 2048], F32R, kind="ExternalInput").ap()
    ident = nc.dram_tensor("ident", [128, 128], BF16, kind="ExternalInput").ap()
    cosT = nc.dram_tensor("cosT", [128, 1024], BF16, kind="ExternalInput").ap()
    sinT = nc.dram_tensor("sinT", [128, 1024], BF16, kind="ExternalInput").ap()
    out = nc.dram_tensor("out", [1024, 2048], F32, kind="ExternalOutput").ap()
    outr = out.rearrange("(tq p) n -> tq p n", p=128)

    EXP = mybir.ActivationFunctionType.Exp

    with tile.TileContext(nc) as tc:
        with tc.tile_pool(name="persist", bufs=1) as persist, \
             tc.tile_pool(name="ylate", bufs=1) as ylate:
            # rotated q (dim1 = group 0-3) and k (4-7), bf16 [chan, T]
            qkT = persist.tile([128, 8, 1024], BF16)
            v_sb = persist.tile([128, 8, 512], BF16)    # [T_k in blk, kb, chan]
            cos_sb = persist.tile([128, 1024], BF16)
            sin_sb = persist.tile([128, 1024], BF16)    # sign-folded
            ones_sb = persist.tile([128, 32], BF16)
            id_sb = persist.tile([128, 128], BF16)
            nc.scalar.dma_start(id_sb, ident)
            nc.scalar.dma_start(cos_sb, cosT)
            nc.scalar.dma_start(sin_sb, sinT)
            nc.vector.memset(ones_sb, 1.0)
            y_tiles = [ylate.tile([128, 1024], F32R, tag=f"y{g}", name=f"y{g}")
                       for g in range(4)]
            wp_half = [ylate.tile([128, 4, 512], F32R, tag=f"wph{n}", name=f"wph{n}")
                       for n in (0, 1)]

            with tc.tile_pool(name="phA", bufs=1) as xpool, \
                 tc.tile_pool(name="wstream", bufs=3) as wsp:
                xt = xpool.tile([128, 16, 1024], F32R)
                # prefetch the first weight chunk ahead of the x bulk so the
                # first projection matmul isn't stuck behind 8MB of x DMA
                wt_first = wsp.tile([128, 16, 128], F32R, tag="wa", name="wt_first")
                nc.sync.dma_start(wt_first, wA[:, 0])
                for ko in range(16):
                    nc.sync.dma_start(xt[:, ko, :], xT[:, ko, :])

                # ---- per-group: project q,k -> pair-swap -> RoPE -> attn ----
                with tc.tile_pool(name="esp", bufs=6) as esp, \
                     tc.tile_pool(name="ztmp", bufs=1) as ztp, \
                     tc.tile_pool(name="qtp", bufs=2) as qtp, \
                     tc.tile_pool(name="psA", bufs=1, space="PSUM") as psa, \
                     tc.tile_pool(name="psS", bufs=4, space="PSUM") as psS, \
                     tc.tile_pool(name="psYZ", bufs=1, space="PSUM") as psYZ:
                    for g in range(4):
                        if g == 2:
                            for n in (0, 1):
                                nc.scalar.dma_start(
                                    wp_half[n], wP[:, :, n * 512:(n + 1) * 512])
                        # project q (mi=g), k (mi=4+g), v^T (mi=8+g)
                        pre = qtp.tile([128, 2, 1024], BF16, tag="pre")
                        swp = qtp.tile([128, 2, 1024], BF16, tag="swp")
                        vtmp = qtp.tile([128, 1024], BF16, tag="vtmp")
                        for j, mi in enumerate((g, 4 + g, 8 + g)):
                            if g == 0 and j == 0:
                                wt = wt_first
                            else:
                                wt = wsp.tile([128, 16, 128], F32R, tag="wa")
                                nc.sync.dma_start(wt, wA[:, mi])
                            ps0 = psa.tile([128, 512], F32, tag="psA0")
                            ps1 = psa.tile([128, 512], F32, tag="psA1")
                            for ko in range(16):
                                st, sp = ko == 0, ko == 15
                                nc.tensor.matmul(ps0, wt[:, ko, :],
                                                 xt[:, ko, 0:512], start=st, stop=sp)
                                nc.tensor.matmul(ps1, wt[:, ko, :],
                                                 xt[:, ko, 512:1024], start=st, stop=sp)
                            dst = vtmp if j == 2 else pre[:, j, :]
                            nc.any.tensor_copy(dst[:, 0:512], ps0)
                            nc.any.tensor_copy(dst[:, 512:1024], ps1)
                        # v natural layout via PE transpose: out = vtmp_blk^T
                        # (regular matmul with identity rhs; contraction over
                        # the channel partitions picks vtmp[n, t]).
                        for kb in range(8):
                            pst = psS.tile([128, 512], F32, tag="pss",
                                           name=f"pst{g}_{kb}")
                            nc.tensor.matmul(pst[:, 0:128],
                                             vtmp[:, kb * 128:(kb + 1) * 128],
                                             id_sb, start=True, stop=True)
                            nc.any.tensor_copy(
                                v_sb[:, kb, g * 128:(g + 1) * 128], pst[:, 0:128])
                        # rotate_half channel pair swap via partition-strided DMA
                        pre_v = pre.rearrange("(a b) j f -> a b j f", b=2)
                        swp_v = swp.rearrange("(a b) j f -> a b j f", b=2)
                        nc.sync.dma_start(swp_v[:, 0], pre_v[:, 1])
                        nc.sync.dma_start(swp_v[:, 1], pre_v[:, 0])
                        # RoPE: qkT = pre*cos + swap*sin_signed
                        for j, dst in enumerate((qkT[:, g, :], qkT[:, 4 + g, :])):
                            nc.vector.tensor_mul(swp[:, j, :], swp[:, j, :], sin_sb)
                            nc.vector.tensor_mul(pre[:, j, :], pre[:, j, :], cos_sb)
                            nc.vector.tensor_add(dst, pre[:, j, :], swp[:, j, :])
                        # attention for this group
                        y_g = y_tiles[g]
                        z_g = ztp.tile([128, 1024], F32, tag="zg")
                        prev_last_exp = None
                        for qc in range(2):
                            q0 = qc * 512
                            nkb = (qc + 1) * 4
                            psy = psYZ.tile([128, 512], F32, tag="psy")
                            psz = psYZ.tile([128, 512], F32, tag="psz")
                            for kb in range(nkb):
                                k0 = kb * 128
                                n0 = max(q0, k0)
                                N = q0 + 512 - n0
                                off = n0 - q0
                                es = esp.tile([128, 4, 512], BF16, tag="es")
                                pss = [psS.tile([128, 512], F32, tag="pss",
                                                name=f"pss{g}_{qc}_{kb}_{h}")
                                       for h in range(4)]
                                s_mms = []
                                for h in range(4):
                                    s_mms.append(nc.tensor.matmul(
                                        pss[h][:, :N],
                                        qkT[32 * h:32 * h + 32, 4 + g, k0:k0 + 128],
                                        qkT[32 * h:32 * h + 32, g, n0:n0 + N],
                                        start=True, stop=True,
                                        tile_position=(32 * h, 0)))
                                if prev_last_exp is not None:
                                    # make all 4 S ready together so they pack
                                    add_dep_helper(s_mms[0].ins, prev_last_exp.ins,
                                                   sync=True, reason="pack S")
                                exps = []
                                for h in range(4):
                                    exps.append(nc.scalar.activation(
                                        es[:, h, :N], pss[h][:, :N], EXP))
                                gate = exps[-1]
                                if k0 >= q0:
                                    sels = []
                                    for h in range(4):
                                        sels.append(nc.gpsimd.affine_select(
                                            es[:, h, 0:128], es[:, h, 0:128],
                                            pattern=[[1, 128]],
                                            channel_multiplier=-1, base=0,
                                            compare_op=mybir.AluOpType.is_ge,
                                            fill=0.0))
                                    gate = sels[-1]
                                prev_last_exp = exps[-1]
                                first_pv = True
                                for h in range(4):
                                    c0 = g * 128 + 32 * h
                                    pv = nc.tensor.matmul(
                                        psy[32 * h:32 * h + 32, off:512],
                                        v_sb[:, kb, c0:c0 + 32],
                                        es[:, h, :N],
                                        start=(kb == 0), stop=(kb == nkb - 1),
                                        tile_position=(0, 32 * h),
                                        skip_group_check=True)
                                    if first_pv:
                                        add_dep_helper(pv.ins, gate.ins,
                                                       sync=True, reason="pack PV")
                                        first_pv = False
                                first_z = True
                                for h in range(4):
                                    z = nc.tensor.matmul(
                                        psz[32 * h:32 * h + 32, off:512],
                                        ones_sb,
                                        es[:, h, :N],
                                        start=(kb == 0), stop=(kb == nkb - 1),
                                        tile_position=(0, 32 * h),
                                        skip_group_check=True)
                                    if first_z:
                                        add_dep_helper(z.ins, gate.ins,
                                                       sync=True, reason="pack Z")
                                        first_z = False
                            nc.vector.tensor_copy(y_g[:, q0:q0 + 512], psy)
                            nc.vector.tensor_copy(z_g[:, q0:q0 + 512], psz)
                        nc.vector.reciprocal(z_g, z_g)
                        nc.vector.tensor_mul(y_g, y_g, z_g)

            # ---- out projection (wP streamed) ----
            with tc.tile_pool(name="phD", bufs=4) as phd, \
                 tc.tile_pool(name="psO", bufs=8, space="PSUM") as psop:
                wp_t = list(wp_half) + [
                    phd.tile([128, 4, 512], F32R, tag=f"wp{n}", name=f"wp{n}")
                    for n in (2, 3)]
                for n in (2, 3):
                    nc.sync.dma_start(wp_t[n], wP[:, :, n * 512:(n + 1) * 512])
                for tq in range(8):
                    pso = [psop.tile([128, 512], F32, tag="pso", name=f"pso{n}")
                           for n in range(4)]
                    for gk in range(4):
                        lhs = y_tiles[gk][:, tq * 128:(tq + 1) * 128]
                        for n in range(4):
                            nc.tensor.matmul(pso[n], lhs, wp_t[n][:, gk, :],
                                             start=(gk == 0), stop=(gk == 3))
                    for n in range(4):
                        o_sb = phd.tile([128, 512], F32, tag="osb")
                        nc.any.tensor_copy(o_sb, pso[n])
                        nc.sync.dma_start(outr[tq][:, n * 512:(n + 1) * 512], o_sb)
    return nc


_NC_CACHE = None


def _host_inputs(x, pos, w_attn, w_proj):
    """Build the 8 per-core input dicts."""
    x = np.asarray(x, dtype=np.float32)
    pos = np.asarray(pos, dtype=np.float32)
    w_attn = np.asarray(w_attn, dtype=np.float32)
    w_proj = np.asarray(w_proj, dtype=np.float32)

    IDENT = np.eye(128, dtype=np.float32).astype(ml_dtypes.bfloat16)
    inv_freq = (1.0 / (10000.0 ** (np.arange(0, H, 2, dtype=np.float32) / H)))
    sinus = pos[:, None] * inv_freq[None, :]              # [T, 32]
    cosT = np.tile(np.cos(sinus).T, (4, 1))               # [128, T]
    sinT = np.tile(np.sin(sinus).T, (4, 1)).copy()
    sinT[0::2, :] *= -1.0                                 # rotate_half signs
    cosT = cosT.astype(ml_dtypes.bfloat16)
    sinT = sinT.astype(ml_dtypes.bfloat16)

    in_maps = []
    for core in range(8):
        b, g = divmod(core, 4)
        hs = slice(g * 512, (g + 1) * 512)
        Wq = (w_attn[:, 0:2048][:, hs] * SCALE).astype(np.float32)
        Wk = w_attn[:, 2048:4096][:, hs]
        Wv = w_attn[:, 4096:6144][:, hs]
        WA = np.concatenate([Wq, Wk, Wv], axis=1)         # [2048, 1536]
        wAr = np.ascontiguousarray(
            WA.reshape(16, 128, 12, 128).transpose(1, 2, 0, 3))  # ki mi ko mc
        wPr = np.ascontiguousarray(
            w_proj[hs, :].reshape(4, 128, 2048).transpose(1, 0, 2))
        xTr = np.ascontiguousarray(
            x[b].T.reshape(16, 128, 1024).transpose(1, 0, 2))
        in_maps.append({
            "xT": xTr, "wA": wAr, "wP": wPr,
            "cosT": cosT, "sinT": sinT, "ident": IDENT,
        })
    return in_maps


def kernel(x, pos, w_attn, w_proj, _trace=False):
    global _NC_CACHE
    _install_patches()
    from concourse.bass_utils import run_bass_kernel_spmd

    if _NC_CACHE is None:
        _NC_CACHE = _build_bass()
    nc = _NC_CACHE
    in_maps = _host_inputs(x, pos, w_attn, w_proj)
    res = run_bass_kernel_spmd(nc, in_maps, core_ids=list(range(8)), trace=_trace)
    outs = [res.results[c]["out"] for c in range(8)]
    full = np.stack([
        outs[0] + outs[1] + outs[2] + outs[3],
        outs[4] + outs[5] + outs[6] + outs[7],
    ]).astype(np.float32)
    kernel.last_results = res
    return full
